# revision 29
# baseline (speedup 1.0000x reference)
"""Sparse (sliding-window + sink) GQA attention block on 8 TRN2 NeuronCores.

Sharding: tensor-parallel over the 64 q-heads -> 8 q-heads (= 1 kv-head
group) per core; x replicated; wo partial outputs summed on host.

All matmuls in bf16 (FWL weight loads, 1 cyc/row at any width); psum f32.
Per-core dataflow:
  A: kv-proj t-loop, then per-et q-proj loops from SBUF-resident packed x
  B: bias-add psum->bf16, RoPE (rotate-half swap via SBUF->SBUF DMA,
     bf16 trig tables with 0.125 q-scale baked), v via PE transpose
  C (pipelined front/back per head): scoresT[j,i] in 4 [128,512] psum
     chunks -> ACT exp psum->bf16 -> 0/1 mask multiply -> split left/right
     pbig matmuls accumulate the window-overlap directly in a [65,1024]
     psum (denom via ones column of v, sink via a spanning rank-1 matmul
     that closes the group) -> reciprocal_approx_fast -> ones-matmul
     broadcast -> scale to at bf16
  D: out[i,dd] partial = sum_et at[et]^T woP, psum->bf16, 8 batched DMAs.
Host: sum bf16 partials (f64) + wo_b.
"""

import numpy as np

B, S, DIM = 1, 1024, 2880
H, HKV, HD = 64, 8, 64
GROUP = H // HKV
WINDOW = 128
THETA = 150000.0
NC = 8
HL = H // NC                 # 8 local q-heads per core
EL = HL * HD                 # 512 local q-dim
DT = (DIM + 127) // 128      # 23 d-tiles (22 full + 64, zero-padded)
NJ = S // 128                # 8 j/i blocks

_cache = {}


def _build_module(taps=False):
    import concourse.bacc as bacc
    import concourse.mybir as mybir
    import concourse.tile as tile

    f32 = mybir.dt.float32
    f32r = mybir.dt.float32r
    bf16 = mybir.dt.bfloat16
    AF = mybir.ActivationFunctionType
    OP = mybir.AluOpType

    nc = bacc.Bacc("TRN2", target_bir_lowering=False, debug=False)

    def din(name, shape, dt=f32):
        return nc.dram_tensor(name, shape, dt, kind="ExternalInput").ap()

    xP = din("xP", [128, DT * 1024], bf16)    # t-major: col 1024t+512sc+c
    wqP = din("wqP", [128, DT * EL], bf16)    # col 512t + e
    wkvP = din("wkvP", [128, DT * 128], bf16)  # col 128t + (k|v)
    woP = din("woP", [128, 4 * DIM], bf16)    # col 2880*et + dd
    qb = din("qb", [128, 4])
    kvb = din("kvb", [128, 1])
    cosq = din("cosq", [128, S], bf16)        # 0.125-scaled
    sinq = din("sinq", [128, S], bf16)        # 0.125-scaled, sign-baked
    cosk = din("cosk", [64, S], bf16)
    sinkt = din("sinkt", [64, S], bf16)
    maskB = din("maskB", [128, 512], bf16)    # 0/1, 256-pattern x2
    esb4 = din("esb4", [128, 4])              # exp(sinks) layouts
    id64 = din("id64", [128, 64], bf16)       # eye(64) stacked twice
    out_d = nc.dram_tensor("out", [S, DIM], bf16, kind="ExternalOutput").ap()
    out2_d = nc.dram_tensor("out2", [S, DIM], bf16,
                            kind="ExternalOutput").ap()
    if taps:
        tap_d = {n: nc.dram_tensor(n, sh, dt, kind="ExternalOutput").ap()
                 for n, sh, dt in [
                     ("d_qbf", [128, 4 * S], bf16), ("d_kb", [128, S], bf16),
                     ("d_vE", [128, NJ * 65], bf16),
                     ("d_eT0", [128, 2048], bf16), ("d_eT1", [128, 2048], bf16),
                     ("d_patv0", [65, S], bf16), ("d_patv1", [65, S], bf16),
                     ("d_rinv0", [1, S], f32), ("d_rinv1", [1, S], f32),
                     ("d_at", [128, 4 * S], bf16)]}

    xsplit = [0, 1, 3, 5, 8, 11, 14, 17, 20, DT]

    with tile.TileContext(nc) as tc:
        import contextlib
        with contextlib.ExitStack() as ctx:
            res = ctx.enter_context(tc.tile_pool(name="res", bufs=1))
            xp_sb = res.tile([128, DT * 1024], bf16, tag="xp")
            wq_sb = res.tile([128, DT * EL], bf16, tag="wq")
            wkv_sb = res.tile([128, DT * 128], bf16, tag="wkv")
            wo_sb = res.tile([128, 4 * DIM], bf16, tag="wo")
            cq_sb = res.tile([128, S], bf16, tag="cq")
            sq_sb = res.tile([128, S], bf16, tag="sq")
            ck_sb = res.tile([64, S], bf16, tag="ck")
            sk_sb = res.tile([64, S], bf16, tag="sk")
            mk_sb = res.tile([128, 512], bf16, tag="mk")
            qb_sb = res.tile([128, 4], f32, tag="qb")
            kvb_sb = res.tile([128, 1], f32, tag="kvb")
            es_sb = res.tile([128, 4], f32, tag="es")
            id_sb = res.tile([128, 64], bf16, tag="id")
            ones_sb = res.tile([1, S], bf16, tag="ones")
            qbf = [res.tile([128, S], bf16, tag=f"qbf{i}", name=f"qbf{i}")
                   for i in range(4)]
            kb_sb = res.tile([128, S], bf16, tag="kb")
            vE_sb = res.tile([128, NJ * 65], bf16, tag="vE")
            at_sb = [res.tile([128, S], bf16, tag=f"at{i}", name=f"at{i}")
                     for i in range(4)]
            dn_sb = [res.tile([128, S], bf16, tag=f"dn{i}", name=f"dn{i}")
                     for i in range(2)]
            rflat = res.tile([1, HL * S], bf16, tag="rflat")

            # wkv + x gate the kv loop: issue them first; wq next;
            # trig/smalls after; wo is deferred until attention starts
            nc.sync.dma_start(wkv_sb[:, :128 * 8], wkvP[:, :128 * 8])
            nc.sync.dma_start(wkv_sb[:, 128 * 8:], wkvP[:, 128 * 8:])
            for c in range(len(xsplit) - 1):
                c0, c1 = 1024 * xsplit[c], 1024 * xsplit[c + 1]
                nc.sync.dma_start(xp_sb[:, c0:c1], xP[:, c0:c1])
            nc.sync.dma_start(kvb_sb[:], kvb[:])
            nc.sync.dma_start(ck_sb[:], cosk[:])
            nc.sync.dma_start(sk_sb[:], sinkt[:])
            nc.sync.dma_start(id_sb[:], id64[:])
            nc.sync.dma_start(qb_sb[:], qb[:])
            w = DT * EL // 4
            for c in range(4):
                nc.sync.dma_start(wq_sb[:, w * c:w * (c + 1)],
                                  wqP[:, w * c:w * (c + 1)])
            nc.sync.dma_start(cq_sb[:], cosq[:])
            nc.sync.dma_start(sq_sb[:], sinq[:])
            nc.sync.dma_start(mk_sb[:], maskB[:])
            nc.sync.dma_start(es_sb[:], esb4[:])
            nc.vector.memset(ones_sb[:], 1.0)
            nc.vector.memset(vE_sb[:], 1.0)

            # ---------------- Phase A1: kv projection + rope + v ----------
            with tc.tile_pool(name="pkv", bufs=1, space="PSUM") as pkv_pool, \
                 tc.tile_pool(name="kvt", bufs=1) as kvt_pool, \
                 tc.tile_pool(name="pvt", bufs=2, space="PSUM") as pvt_pool:
                pkv = pkv_pool.tile([128, S], f32, tag="pkv")
                for t in range(DT):
                    for sc in range(2):
                        nc.tensor.matmul(
                            pkv[:, 512 * sc:512 * (sc + 1)],
                            wkv_sb[:, 128 * t:128 * (t + 1)],
                            xp_sb[:, 1024 * t + 512 * sc:
                                  1024 * t + 512 * (sc + 1)],
                            start=(t == 0), stop=(t == DT - 1))
                kv_b = kvt_pool.tile([128, S], bf16, tag="kvb16")
                nc.vector.tensor_scalar_add(kv_b[:], pkv[:], kvb_sb[:, 0:1])
                # k rope (rows 0:64): swap halves via SBUF->SBUF DMA
                ksw = kvt_pool.tile([64, S], bf16, tag="ksw")
                nc.scalar.dma_start(ksw[0:32, :], kv_b[32:64, :])
                nc.scalar.dma_start(ksw[32:64, :], kv_b[0:32, :])
                kt1 = kvt_pool.tile([64, S], bf16, tag="kt1")
                kt2 = kvt_pool.tile([64, S], bf16, tag="kt2")
                nc.vector.tensor_tensor(kt1[:], ksw[:], sk_sb[:], op=OP.mult)
                nc.vector.tensor_tensor(kt2[:], kv_b[0:64, :], ck_sb[:],
                                        op=OP.mult)
                nc.vector.tensor_tensor(kb_sb[0:64, :], kt1[:], kt2[:],
                                        op=OP.add)
                # replicate kT at partition base 64 for odd heads
                nc.scalar.dma_start(kb_sb[64:128, :], kb_sb[0:64, :])
                # v transposes: [64,128] blocks -> [128 j, 64 hd] bf16
                for j in range(NJ):
                    pvt = pvt_pool.tile([128, 64], bf16, tag="pvt")
                    nc.tensor.transpose(
                        pvt[:], kv_b[64:128, 128 * j:128 * (j + 1)],
                        id_sb[64:128, :])
                    nc.vector.tensor_copy(vE_sb[:, 65 * j:65 * j + 64],
                                          pvt[:])

            # ------- Phase A2+B+C: q proj + rope, pipelined attention -----
            with tc.tile_pool(name="psc", bufs=2, space="PSUM") as sc_pool, \
                 tc.tile_pool(name="pat", bufs=2, space="PSUM") as at_pool, \
                 tc.tile_pool(name="eT", bufs=2) as et_pool, \
                 tc.tile_pool(name="er", bufs=2) as er_pool, \
                 tc.tile_pool(name="pv", bufs=8) as pv_pool, \
                 tc.tile_pool(name="nrm", bufs=1) as nm_pool, \
                 tc.tile_pool(name="rbd", bufs=2) as rb_pool:

                aux_ctx = contextlib.ExitStack()
                pq_pool = aux_ctx.enter_context(
                    tc.tile_pool(name="pq", bufs=2, space="PSUM"))
                qt_pool = aux_ctx.enter_context(
                    tc.tile_pool(name="qt", bufs=2))

                def q_proj(et):
                    qt = qt_pool.tile([128, S], bf16, tag="qt")
                    for sc in range(2):
                        pq = pq_pool.tile([128, 512], f32, tag="pq")
                        for t in range(DT):
                            nc.tensor.matmul(
                                pq[:],
                                wq_sb[:, EL * t + 128 * et:
                                      EL * t + 128 * (et + 1)],
                                xp_sb[:, 1024 * t + 512 * sc:
                                      1024 * t + 512 * (sc + 1)],
                                start=(t == 0), stop=(t == DT - 1))
                        nc.vector.tensor_scalar_add(
                            qt[:, 512 * sc:512 * (sc + 1)], pq[:],
                            qb_sb[:, et:et + 1])
                    qsw = qt_pool.tile([128, S], bf16, tag="qsw")
                    nc.scalar.dma_start(qsw[0:32, :], qt[32:64, :])
                    nc.scalar.dma_start(qsw[32:64, :], qt[0:32, :])
                    nc.scalar.dma_start(qsw[64:96, :], qt[96:128, :])
                    nc.scalar.dma_start(qsw[96:128, :], qt[64:96, :])
                    t1 = qt_pool.tile([128, S], bf16, tag="t1")
                    t2 = qt_pool.tile([128, S], bf16, tag="t2")
                    nc.vector.tensor_tensor(t1[:], qsw[:], sq_sb[:],
                                            op=OP.mult)
                    nc.vector.tensor_tensor(t2[:], qt[:], cq_sb[:],
                                            op=OP.mult)
                    nc.vector.tensor_tensor(qbf[et][:], t1[:], t2[:],
                                            op=OP.add)

                state = {}
                wo_loaded = []

                def head_front(h):
                    if not wo_loaded:
                        nc.sync.dma_start(wo_sb[:], woP[:])
                        wo_loaded.append(True)
                    qt = qbf[h // 2]
                    r0 = 64 * (h % 2)
                    eT = et_pool.tile([128, 2048], bf16, tag="eT",
                                      name=f"eT{h}")
                    for ch in range(4):
                        ncols = 512 if ch < 3 else 384
                        ps = sc_pool.tile([128, 512], f32, tag="ps")
                        for Jl in range(2):
                            J = 2 * ch + Jl
                            ni = 256 if J < NJ - 1 else 128
                            nc.tensor.matmul(
                                ps[:, 256 * Jl:256 * Jl + ni],
                                kb_sb[r0:r0 + 64, 128 * J:128 * (J + 1)],
                                qt[r0:r0 + 64, 128 * J:128 * J + ni],
                                start=True, stop=True)
                        er = er_pool.tile([128, 512], bf16, tag="er")
                        nc.scalar.activation(
                            er[:, :ncols], ps[:, :ncols], AF.Exp)
                        eng = nc.gpsimd if ch == 3 else nc.vector
                        eng.tensor_tensor(
                            eT[:, 512 * ch:512 * ch + ncols],
                            er[:, :ncols], mk_sb[:, :ncols], op=OP.mult)
                    pat = at_pool.tile([128, S], f32, tag="pat",
                                       name=f"pat{h}")
                    for I in range(NJ):
                        if I > 0:
                            nc.tensor.matmul(
                                pat[0:65, 128 * I:128 * (I + 1)],
                                vE_sb[:, 65 * (I - 1):65 * I],
                                eT[:, 256 * (I - 1) + 128:256 * I],
                                start=True, stop=False)
                        nc.tensor.matmul(
                            pat[0:65, 128 * I:128 * (I + 1)],
                            vE_sb[:, 65 * I:65 * (I + 1)],
                            eT[:, 256 * I:256 * I + 128],
                            start=(I == 0), stop=True)
                    patv = pv_pool.tile([65, S], bf16, tag="patv",
                                        name=f"patv{h}")
                    nc.vector.tensor_copy(patv[:], pat[0:65, :])
                    rbd = None
                    if h < 6:
                        dr = 32 * (h % 4)
                        nc.scalar.dma_start(dn_sb[h // 4][dr:dr + 1, :],
                                            patv[64:65, :])
                    else:
                        rbd = norm_direct(h, pat)
                    if taps and h < 2:
                        nc.sync.dma_start(tap_d[f"d_eT{h}"][:], eT[:])
                        nc.sync.dma_start(tap_d[f"d_patv{h}"][:], patv[:])
                    state[h] = (patv, r0, rbd)

                def norm_batch(b, nh):
                    # 1/(denom+es) for nh heads at once (rows 32*hl)
                    tmp = nm_pool.tile([128, S], f32, tag="tmpf")
                    rvf = nm_pool.tile([128, S], f32, tag="rvf")
                    rvb = nm_pool.tile([128, S], bf16, tag="rvb")
                    nc.vector.tensor_scalar_add(tmp[:], dn_sb[b][:],
                                                es_sb[:, b:b + 1])
                    nc.vector.reciprocal_approx_fast(rvf[:], tmp[:])
                    nc.vector.tensor_copy(rvb[:], rvf[:])
                    for hl in range(nh):
                        h = 4 * b + hl
                        nc.scalar.dma_start(
                            rflat[0:1, S * h:S * (h + 1)],
                            rvb[32 * hl:32 * hl + 1, :])
                        if taps and h < 2:
                            nc.sync.dma_start(
                                tap_d[f"d_rinv{h}"][:],
                                rvf[32 * hl:32 * hl + 1, :])

                def norm_direct(h, pat):
                    # short DVE-only chain for the last heads
                    dnd = nm_pool.tile([128, S], f32, tag="tmpf",
                                       name=f"dnd{h}")
                    nc.vector.tensor_scalar_add(
                        dnd[0:1, :], pat[64:65, :],
                        es_sb[64:65, h - 4:h - 3])
                    rvd = nm_pool.tile([128, S], f32, tag="rvf",
                                       name=f"rvd{h}")
                    nc.vector.reciprocal_approx_fast(rvd[0:1, :],
                                                     dnd[0:1, :])
                    rbd = rb_pool.tile([1, S], bf16, tag="rbd",
                                       name=f"rbd{h}")
                    nc.vector.tensor_copy(rbd[:], rvd[0:1, :])
                    return rbd

                def head_back(h):
                    patv, r0, rbd = state.pop(h)
                    for half in range(2):
                        prb = sc_pool.tile([128, 512], f32, tag="ps",
                                           name=f"prb{h}_{half}")
                        rsrc = (rflat[0:1, S * h + 512 * half:
                                      S * h + 512 * (half + 1)]
                                if rbd is None else
                                rbd[0:1, 512 * half:512 * (half + 1)])
                        nc.tensor.matmul(
                            prb[r0:r0 + 64, :],
                            ones_sb[0:1, 0:64],
                            rsrc,
                            start=True, stop=True)
                        nc.vector.tensor_tensor(
                            at_sb[h // 2][r0:r0 + 64,
                                          512 * half:512 * (half + 1)],
                            patv[0:64, 512 * half:512 * (half + 1)],
                            prb[r0:r0 + 64, :], op=OP.mult)

                NDD = 6
                DDC = DIM // NDD  # 480
                dh2_pools = []

                def d_half(ets, dst, its, po_pool, ob_pool, cnt=[0]):
                    for it in its:
                        ob = ob_pool.tile([128, DIM], bf16, tag="ob")
                        for dd in range(NDD):
                            po = po_pool.tile([128, DDC], f32, tag="po")
                            for i, et in enumerate(ets):
                                nc.tensor.matmul(
                                    po[:],
                                    at_sb[et][:, 128 * it:128 * (it + 1)],
                                    wo_sb[:, DIM * et + DDC * dd:
                                          DIM * et + DDC * (dd + 1)],
                                    start=(i == 0), stop=(i == len(ets) - 1))
                            cnt[0] += 1
                            if cnt[0] % 2 == 0:
                                nc.scalar.activation(
                                    ob[:, DDC * dd:DDC * (dd + 1)], po[:],
                                    AF.Copy)
                            else:
                                nc.vector.tensor_copy(
                                    ob[:, DDC * dd:DDC * (dd + 1)], po[:])
                        nc.sync.dma_start(
                            dst[128 * it:128 * (it + 1), :], ob[:])

                q_proj(0)
                q_proj(1)
                head_front(0)
                head_front(1)
                q_proj(2)
                head_front(2)
                head_front(3)
                norm_batch(0, 4)
                q_proj(3)
                aux_ctx.close()  # free pq psum + qt sbuf for D-half pools
                aux2 = contextlib.ExitStack()
                po_pool = aux2.enter_context(
                    tc.tile_pool(name="po", bufs=2, space="PSUM"))
                ob_pool = aux2.enter_context(tc.tile_pool(name="ob", bufs=2))
                head_front(4)
                head_front(5)
                norm_batch(1, 2)
                head_back(0)
                head_back(1)
                head_back(2)
                head_back(3)
                head_front(6)
                head_back(4)
                d_half((0, 1), out2_d, range(0, 4), po_pool, ob_pool)
                head_front(7)
                head_back(5)
                d_half((0, 1), out2_d, range(4, NJ), po_pool, ob_pool)
                head_back(6)
                head_back(7)
                aux2.close()
                dh2_pools.append(d_half)


            with tc.tile_pool(name="po2", bufs=4, space="PSUM") as po2, \
                 tc.tile_pool(name="ob2", bufs=2) as ob2:
                dh2_pools[0]((2, 3), out_d, range(NJ), po2, ob2)
            if taps:
                for i in range(4):
                    nc.sync.dma_start(tap_d["d_qbf"][:, S * i:S * (i + 1)],
                                      qbf[i][:])
                    nc.sync.dma_start(tap_d["d_at"][:, S * i:S * (i + 1)],
                                      at_sb[i][:])
                nc.sync.dma_start(tap_d["d_kb"][:], kb_sb[:])
                nc.sync.dma_start(tap_d["d_vE"][:], vE_sb[:])

    nc.compile()
    return nc


def _host_prep(x, wq_w, wq_b, wk_w, wk_b, wv_w, wv_b, wo_w, wo_b, sinks):
    """Build per-core input maps (host-side sharding + layout prep)."""
    import ml_dtypes
    f = np.float32
    bf = ml_dtypes.bfloat16
    xm = x.reshape(S, DIM).astype(f)

    xP = np.zeros((128, DT * 1024), bf)
    for t in range(DT):
        dp = min(128, DIM - 128 * t)
        blk = xm[:, 128 * t:128 * t + dp].T.astype(bf)     # [dp, S]
        xP[:dp, 1024 * t:1024 * t + 512] = blk[:, 0:512]
        xP[:dp, 1024 * t + 512:1024 * (t + 1)] = blk[:, 512:1024]

    half = HD // 2
    inv_freq = 1.0 / (THETA ** (np.arange(half, dtype=np.float64) * 2.0 / HD))
    ang = np.arange(S, dtype=np.float64)[:, None] * inv_freq
    cos_t = np.cos(ang).T
    sin_t = np.sin(ang).T
    cos64 = np.concatenate([cos_t, cos_t], 0)              # [64, S]
    sin64 = np.concatenate([-sin_t, sin_t], 0)
    scale = HD ** -0.5
    cosq = (np.concatenate([cos64, cos64], 0) * scale).astype(bf)
    sinq = (np.concatenate([sin64, sin64], 0) * scale).astype(bf)
    cosk = cos64.astype(bf)
    sinkt = sin64.astype(bf)

    jj = np.arange(128)[:, None]
    ii = np.arange(256)[None, :]
    allow_l = (jj <= ii) & (ii < 128)
    allow_r = (ii >= 128) & (jj > ii - 128)
    mask256 = (allow_l | allow_r).astype(bf)
    maskB = np.concatenate([mask256, mask256], 1)          # [128, 512]

    id64 = np.tile(np.eye(64), (2, 1)).astype(bf)

    def tile_T(w, E):  # [E, DIM] -> tiled transposed [128, DT*E] bf16
        out = np.zeros((128, DT * E), bf)
        for t in range(DT):
            dp = min(128, DIM - 128 * t)
            out[:dp, E * t:E * (t + 1)] = \
                w[:, 128 * t:128 * t + dp].T.astype(bf)
        return out

    in_maps = []
    for c in range(NC):
        wq_c = wq_w[EL * c:EL * (c + 1)]                  # [512, 2880]
        wkv_c = np.concatenate([wk_w[HD * c:HD * (c + 1)],
                                wv_w[HD * c:HD * (c + 1)]], 0)  # [128, 2880]
        wo_c = np.ascontiguousarray(wo_w[:, EL * c:EL * (c + 1)].T)  # [512,2880]
        woP = np.zeros((128, 4 * DIM), bf)
        for et in range(4):
            woP[:, DIM * et:DIM * (et + 1)] = \
                wo_c[128 * et:128 * (et + 1)].astype(bf)
        es = np.exp(sinks[HL * c:HL * (c + 1)].astype(np.float64)).astype(f)
        esb4 = np.zeros((128, 4), f)
        for h in range(6):
            esb4[32 * (h % 4), h // 4] = es[h]
        esb4[64, 2] = es[6]
        esb4[64, 3] = es[7]
        in_maps.append({
            "xP": xP,
            "wqP": tile_T(wq_c, EL),
            "wkvP": tile_T(wkv_c, 128),
            "woP": woP,
            "qb": np.ascontiguousarray(
                wq_b[EL * c:EL * (c + 1)].reshape(4, 128).T).astype(f),
            "kvb": np.ascontiguousarray(np.concatenate(
                [wk_b[HD * c:HD * (c + 1)],
                 wv_b[HD * c:HD * (c + 1)]]).reshape(1, 128).T).astype(f),
            "cosq": cosq, "sinq": sinq, "cosk": cosk, "sinkt": sinkt,
            "maskB": maskB, "esb4": esb4, "id64": id64,
        })
    return in_maps


def run_on_hw(inputs, trace=False, taps=False, **kw):
    from concourse import bass_utils
    if "nc" not in _cache:
        _cache["nc"] = _build_module(taps=taps)
    in_maps = _host_prep(**inputs)
    res = bass_utils.run_bass_kernel_spmd(
        _cache["nc"], in_maps, core_ids=list(range(NC)), trace=trace, **kw)
    out = np.zeros((S, DIM), np.float64)
    for c in range(NC):
        out += np.asarray(res.results[c]["out"], dtype=np.float64)
        out += np.asarray(res.results[c]["out2"], dtype=np.float64)
    out = (out + inputs["wo_b"].astype(np.float64)).astype(np.float32)
    return out.reshape(B, S, DIM), res


def kernel(**inputs) -> np.ndarray:
    out, _ = run_on_hw(inputs, trace=False)
    return out


# revision 30
# speedup vs baseline: 1.1403x; 1.1403x over previous
"""Sparse (sliding-window + sink) GQA attention block on 8 TRN2 NeuronCores.

Sharding: tensor-parallel over the 64 q-heads -> 8 q-heads (= 1 kv-head
group) per core; x replicated; wo partial outputs summed on host.

All matmuls in bf16 (FWL weight loads, 1 cyc/row at any width); psum f32.
Per-core dataflow:
  A: kv-proj t-loop, then per-et q-proj loops from SBUF-resident packed x
  B: bias-add psum->bf16, RoPE (rotate-half swap via SBUF->SBUF DMA,
     bf16 trig tables with 0.125 q-scale baked), v via PE transpose
  C (pipelined front/back per head): scoresT[j,i] in 4 [128,512] psum
     chunks -> ACT exp psum->bf16 -> 0/1 mask multiply -> split left/right
     pbig matmuls accumulate the window-overlap directly in a [65,1024]
     psum (denom via ones column of v, sink via a spanning rank-1 matmul
     that closes the group) -> reciprocal_approx_fast -> ones-matmul
     broadcast -> scale to at bf16
  D: out[i,dd] partial = sum_et at[et]^T woP, psum->bf16, 8 batched DMAs.
Host: sum bf16 partials (f64) + wo_b.
"""

import numpy as np

B, S, DIM = 1, 1024, 2880
H, HKV, HD = 64, 8, 64
GROUP = H // HKV
WINDOW = 128
THETA = 150000.0
NC = 8
HL = H // NC                 # 8 local q-heads per core
EL = HL * HD                 # 512 local q-dim
DT = (DIM + 127) // 128      # 23 d-tiles (22 full + 64, zero-padded)
NJ = S // 128                # 8 j/i blocks

_cache = {}


def _build_module(taps=False):
    import concourse.bacc as bacc
    import concourse.mybir as mybir
    import concourse.tile as tile

    f32 = mybir.dt.float32
    f32r = mybir.dt.float32r
    bf16 = mybir.dt.bfloat16
    AF = mybir.ActivationFunctionType
    OP = mybir.AluOpType

    nc = bacc.Bacc("TRN2", target_bir_lowering=False, debug=False)

    def din(name, shape, dt=f32):
        return nc.dram_tensor(name, shape, dt, kind="ExternalInput").ap()

    xP = din("xP", [128, DT * 1024], bf16)    # t-major: col 1024t+512sc+c
    wqP = din("wqP", [128, DT * EL], bf16)    # col 512t + e
    wkvP = din("wkvP", [128, DT * 128], bf16)  # col 128t + (k|v)
    woP = din("woP", [128, 4 * DIM], bf16)    # col 2880*et + dd
    qb = din("qb", [128, 4])
    kvb = din("kvb", [128, 1])
    cosq = din("cosq", [128, S], bf16)        # 0.125-scaled
    sinq = din("sinq", [128, S], bf16)        # 0.125-scaled, sign-baked
    cosk = din("cosk", [64, S], bf16)
    sinkt = din("sinkt", [64, S], bf16)
    maskB = din("maskB", [128, 512], bf16)    # 0/1, 256-pattern x2
    esb4 = din("esb4", [128, 4])              # exp(sinks) layouts
    id64 = din("id64", [128, 64], bf16)       # eye(64) stacked twice
    out_d = nc.dram_tensor("out", [S, DIM], bf16, kind="ExternalOutput").ap()
    if taps:
        tap_d = {n: nc.dram_tensor(n, sh, dt, kind="ExternalOutput").ap()
                 for n, sh, dt in [
                     ("d_qbf", [128, 4 * S], bf16), ("d_kb", [128, S], bf16),
                     ("d_vE", [128, NJ * 65], bf16),
                     ("d_eT0", [128, 2048], bf16), ("d_eT1", [128, 2048], bf16),
                     ("d_patv0", [65, S], bf16), ("d_patv1", [65, S], bf16),
                     ("d_rinv0", [1, S], f32), ("d_rinv1", [1, S], f32),
                     ("d_at", [128, 4 * S], bf16)]}

    xsplit = [0, 1, 3, 5, 8, 11, 14, 17, 20, DT]

    with tile.TileContext(nc) as tc:
        import contextlib
        with contextlib.ExitStack() as ctx:
            res = ctx.enter_context(tc.tile_pool(name="res", bufs=1))
            xp_sb = res.tile([128, DT * 1024], bf16, tag="xp")
            wq_sb = res.tile([128, DT * EL], bf16, tag="wq")
            wkv_sb = res.tile([128, DT * 128], bf16, tag="wkv")
            wo_sb = res.tile([128, 4 * DIM], bf16, tag="wo")
            cq_sb = res.tile([128, S], bf16, tag="cq")
            sq_sb = res.tile([128, S], bf16, tag="sq")
            ck_sb = res.tile([64, S], bf16, tag="ck")
            sk_sb = res.tile([64, S], bf16, tag="sk")
            mk_sb = res.tile([128, 512], bf16, tag="mk")
            qb_sb = res.tile([128, 4], f32, tag="qb")
            kvb_sb = res.tile([128, 1], f32, tag="kvb")
            es_sb = res.tile([128, 4], f32, tag="es")
            id_sb = res.tile([128, 64], bf16, tag="id")
            ones_sb = res.tile([1, S], bf16, tag="ones")
            qbf = [res.tile([128, S], bf16, tag=f"qbf{i}", name=f"qbf{i}")
                   for i in range(4)]
            kb_sb = res.tile([128, S], bf16, tag="kb")
            vE_sb = res.tile([128, NJ * 65], bf16, tag="vE")
            at_sb = [res.tile([128, S], bf16, tag=f"at{i}", name=f"at{i}")
                     for i in range(4)]
            dn_sb = [res.tile([128, S], bf16, tag=f"dn{i}", name=f"dn{i}")
                     for i in range(2)]
            rflat = res.tile([1, HL * S], bf16, tag="rflat")

            # wkv + x gate the kv loop: issue them first; wq next;
            # trig/smalls after; wo is deferred until attention starts
            nc.sync.dma_start(wkv_sb[:, :128 * 8], wkvP[:, :128 * 8])
            nc.sync.dma_start(wkv_sb[:, 128 * 8:], wkvP[:, 128 * 8:])
            for c in range(len(xsplit) - 1):
                c0, c1 = 1024 * xsplit[c], 1024 * xsplit[c + 1]
                nc.sync.dma_start(xp_sb[:, c0:c1], xP[:, c0:c1])
            nc.sync.dma_start(kvb_sb[:], kvb[:])
            nc.sync.dma_start(ck_sb[:], cosk[:])
            nc.sync.dma_start(sk_sb[:], sinkt[:])
            nc.sync.dma_start(id_sb[:], id64[:])
            nc.sync.dma_start(qb_sb[:], qb[:])
            w = DT * EL // 4
            for c in range(4):
                nc.sync.dma_start(wq_sb[:, w * c:w * (c + 1)],
                                  wqP[:, w * c:w * (c + 1)])
            nc.sync.dma_start(cq_sb[:], cosq[:])
            nc.sync.dma_start(sq_sb[:], sinq[:])
            nc.sync.dma_start(mk_sb[:], maskB[:])
            nc.sync.dma_start(es_sb[:], esb4[:])
            nc.vector.memset(ones_sb[:], 1.0)
            nc.vector.memset(vE_sb[:], 1.0)

            # ---------------- Phase A1: kv projection + rope + v ----------
            with tc.tile_pool(name="pkv", bufs=1, space="PSUM") as pkv_pool, \
                 tc.tile_pool(name="kvt", bufs=1) as kvt_pool, \
                 tc.tile_pool(name="pvt", bufs=2, space="PSUM") as pvt_pool:
                pkv = pkv_pool.tile([128, S], f32, tag="pkv")
                for t in range(DT):
                    for sc in range(2):
                        nc.tensor.matmul(
                            pkv[:, 512 * sc:512 * (sc + 1)],
                            wkv_sb[:, 128 * t:128 * (t + 1)],
                            xp_sb[:, 1024 * t + 512 * sc:
                                  1024 * t + 512 * (sc + 1)],
                            start=(t == 0), stop=(t == DT - 1))
                kv_b = kvt_pool.tile([128, S], bf16, tag="kvb16")
                nc.vector.tensor_scalar_add(kv_b[:], pkv[:], kvb_sb[:, 0:1])
                # k rope (rows 0:64): swap halves via SBUF->SBUF DMA
                ksw = kvt_pool.tile([64, S], bf16, tag="ksw")
                nc.scalar.dma_start(ksw[0:32, :], kv_b[32:64, :])
                nc.scalar.dma_start(ksw[32:64, :], kv_b[0:32, :])
                kt1 = kvt_pool.tile([64, S], bf16, tag="kt1")
                kt2 = kvt_pool.tile([64, S], bf16, tag="kt2")
                nc.vector.tensor_tensor(kt1[:], ksw[:], sk_sb[:], op=OP.mult)
                nc.vector.tensor_tensor(kt2[:], kv_b[0:64, :], ck_sb[:],
                                        op=OP.mult)
                nc.vector.tensor_tensor(kb_sb[0:64, :], kt1[:], kt2[:],
                                        op=OP.add)
                # replicate kT at partition base 64 for odd heads
                nc.scalar.dma_start(kb_sb[64:128, :], kb_sb[0:64, :])
                # v transposes: [64,128] blocks -> [128 j, 64 hd] bf16
                for j in range(NJ):
                    pvt = pvt_pool.tile([128, 64], bf16, tag="pvt")
                    nc.tensor.transpose(
                        pvt[:], kv_b[64:128, 128 * j:128 * (j + 1)],
                        id_sb[64:128, :])
                    nc.vector.tensor_copy(vE_sb[:, 65 * j:65 * j + 64],
                                          pvt[:])

            # ------- Phase A2+B+C: q proj + rope, pipelined attention -----
            with tc.tile_pool(name="psc", bufs=2, space="PSUM") as sc_pool, \
                 tc.tile_pool(name="pat", bufs=2, space="PSUM") as at_pool, \
                 tc.tile_pool(name="eT", bufs=2) as et_pool, \
                 tc.tile_pool(name="er", bufs=2) as er_pool, \
                 tc.tile_pool(name="pv", bufs=8) as pv_pool, \
                 tc.tile_pool(name="nrm", bufs=1) as nm_pool, \
                 tc.tile_pool(name="rbd", bufs=2) as rb_pool:

                aux_ctx = contextlib.ExitStack()
                pq_pool = aux_ctx.enter_context(
                    tc.tile_pool(name="pq", bufs=2, space="PSUM"))
                qt_pool = aux_ctx.enter_context(
                    tc.tile_pool(name="qt", bufs=2))

                def q_proj(et):
                    qt = qt_pool.tile([128, S], bf16, tag="qt")
                    for sc in range(2):
                        pq = pq_pool.tile([128, 512], f32, tag="pq")
                        for t in range(DT):
                            nc.tensor.matmul(
                                pq[:],
                                wq_sb[:, EL * t + 128 * et:
                                      EL * t + 128 * (et + 1)],
                                xp_sb[:, 1024 * t + 512 * sc:
                                      1024 * t + 512 * (sc + 1)],
                                start=(t == 0), stop=(t == DT - 1))
                        nc.vector.tensor_scalar_add(
                            qt[:, 512 * sc:512 * (sc + 1)], pq[:],
                            qb_sb[:, et:et + 1])
                    qsw = qt_pool.tile([128, S], bf16, tag="qsw")
                    nc.scalar.dma_start(qsw[0:32, :], qt[32:64, :])
                    nc.scalar.dma_start(qsw[32:64, :], qt[0:32, :])
                    nc.scalar.dma_start(qsw[64:96, :], qt[96:128, :])
                    nc.scalar.dma_start(qsw[96:128, :], qt[64:96, :])
                    t1 = qt_pool.tile([128, S], bf16, tag="t1")
                    t2 = qt_pool.tile([128, S], bf16, tag="t2")
                    nc.vector.tensor_tensor(t1[:], qsw[:], sq_sb[:],
                                            op=OP.mult)
                    nc.vector.tensor_tensor(t2[:], qt[:], cq_sb[:],
                                            op=OP.mult)
                    nc.vector.tensor_tensor(qbf[et][:], t1[:], t2[:],
                                            op=OP.add)

                state = {}
                wo_loaded = []

                def head_front(h):
                    if not wo_loaded:
                        nc.sync.dma_start(wo_sb[:], woP[:])
                        wo_loaded.append(True)
                    qt = qbf[h // 2]
                    r0 = 64 * (h % 2)
                    eT = et_pool.tile([128, 2048], bf16, tag="eT",
                                      name=f"eT{h}")
                    for ch in range(4):
                        ncols = 512 if ch < 3 else 384
                        ps = sc_pool.tile([128, 512], f32, tag="ps")
                        for Jl in range(2):
                            J = 2 * ch + Jl
                            ni = 256 if J < NJ - 1 else 128
                            nc.tensor.matmul(
                                ps[:, 256 * Jl:256 * Jl + ni],
                                kb_sb[r0:r0 + 64, 128 * J:128 * (J + 1)],
                                qt[r0:r0 + 64, 128 * J:128 * J + ni],
                                start=True, stop=True)
                        er = er_pool.tile([128, 512], bf16, tag="er")
                        nc.scalar.activation(
                            er[:, :ncols], ps[:, :ncols], AF.Exp)
                        eng = nc.gpsimd if ch == 3 else nc.vector
                        eng.tensor_tensor(
                            eT[:, 512 * ch:512 * ch + ncols],
                            er[:, :ncols], mk_sb[:, :ncols], op=OP.mult)
                    pat = at_pool.tile([128, S], f32, tag="pat",
                                       name=f"pat{h}")
                    for I in range(NJ):
                        if I > 0:
                            nc.tensor.matmul(
                                pat[0:65, 128 * I:128 * (I + 1)],
                                vE_sb[:, 65 * (I - 1):65 * I],
                                eT[:, 256 * (I - 1) + 128:256 * I],
                                start=True, stop=False)
                        nc.tensor.matmul(
                            pat[0:65, 128 * I:128 * (I + 1)],
                            vE_sb[:, 65 * I:65 * (I + 1)],
                            eT[:, 256 * I:256 * I + 128],
                            start=(I == 0), stop=True)
                    patv = pv_pool.tile([65, S], bf16, tag="patv",
                                        name=f"patv{h}")
                    nc.vector.tensor_copy(patv[:], pat[0:65, :])
                    rbd = None
                    if h < 6:
                        dr = 32 * (h % 4)
                        nc.scalar.dma_start(dn_sb[h // 4][dr:dr + 1, :],
                                            patv[64:65, :])
                    else:
                        rbd = norm_direct(h, pat)
                    if taps and h < 2:
                        nc.sync.dma_start(tap_d[f"d_eT{h}"][:], eT[:])
                        nc.sync.dma_start(tap_d[f"d_patv{h}"][:], patv[:])
                    state[h] = (patv, r0, rbd)

                def norm_batch(b, nh):
                    # 1/(denom+es) for nh heads at once (rows 32*hl)
                    tmp = nm_pool.tile([128, S], f32, tag="tmpf")
                    rvf = nm_pool.tile([128, S], f32, tag="rvf")
                    rvb = nm_pool.tile([128, S], bf16, tag="rvb")
                    nc.vector.tensor_scalar_add(tmp[:], dn_sb[b][:],
                                                es_sb[:, b:b + 1])
                    nc.vector.reciprocal_approx_fast(rvf[:], tmp[:])
                    nc.vector.tensor_copy(rvb[:], rvf[:])
                    for hl in range(nh):
                        h = 4 * b + hl
                        nc.scalar.dma_start(
                            rflat[0:1, S * h:S * (h + 1)],
                            rvb[32 * hl:32 * hl + 1, :])
                        if taps and h < 2:
                            nc.sync.dma_start(
                                tap_d[f"d_rinv{h}"][:],
                                rvf[32 * hl:32 * hl + 1, :])

                def norm_direct(h, pat):
                    # short DVE-only chain for the last heads
                    dnd = nm_pool.tile([128, S], f32, tag="tmpf",
                                       name=f"dnd{h}")
                    nc.vector.tensor_scalar_add(
                        dnd[0:1, :], pat[64:65, :],
                        es_sb[64:65, h - 4:h - 3])
                    rvd = nm_pool.tile([128, S], f32, tag="rvf",
                                       name=f"rvd{h}")
                    nc.vector.reciprocal_approx_fast(rvd[0:1, :],
                                                     dnd[0:1, :])
                    rbd = rb_pool.tile([1, S], bf16, tag="rbd",
                                       name=f"rbd{h}")
                    nc.vector.tensor_copy(rbd[:], rvd[0:1, :])
                    return rbd

                def head_back(h):
                    patv, r0, rbd = state.pop(h)
                    for half in range(2):
                        prb = sc_pool.tile([128, 512], f32, tag="ps",
                                           name=f"prb{h}_{half}")
                        rsrc = (rflat[0:1, S * h + 512 * half:
                                      S * h + 512 * (half + 1)]
                                if rbd is None else
                                rbd[0:1, 512 * half:512 * (half + 1)])
                        nc.tensor.matmul(
                            prb[r0:r0 + 64, :],
                            ones_sb[0:1, 0:64],
                            rsrc,
                            start=True, stop=True)
                        nc.vector.tensor_tensor(
                            at_sb[h // 2][r0:r0 + 64,
                                          512 * half:512 * (half + 1)],
                            patv[0:64, 512 * half:512 * (half + 1)],
                            prb[r0:r0 + 64, :], op=OP.mult)

                NDD = 6
                DDC = DIM // NDD  # 480
                dh2_pools = []

                def d_half(ets, dst, its, po_pool, ob_pool, cnt=[0]):
                    for it in its:
                        ob = ob_pool.tile([128, DIM], bf16, tag="ob")
                        for dd in range(NDD):
                            po = po_pool.tile([128, DDC], f32, tag="po")
                            for i, et in enumerate(ets):
                                nc.tensor.matmul(
                                    po[:],
                                    at_sb[et][:, 128 * it:128 * (it + 1)],
                                    wo_sb[:, DIM * et + DDC * dd:
                                          DIM * et + DDC * (dd + 1)],
                                    start=(i == 0), stop=(i == len(ets) - 1))
                            cnt[0] += 1
                            if cnt[0] % 2 == 0:
                                nc.scalar.activation(
                                    ob[:, DDC * dd:DDC * (dd + 1)], po[:],
                                    AF.Copy)
                            else:
                                nc.vector.tensor_copy(
                                    ob[:, DDC * dd:DDC * (dd + 1)], po[:])
                        nc.sync.dma_start(
                            dst[128 * it:128 * (it + 1), :], ob[:])

                q_proj(0)
                q_proj(1)
                head_front(0)
                head_front(1)
                q_proj(2)
                head_front(2)
                head_front(3)
                norm_batch(0, 4)
                q_proj(3)
                aux_ctx.close()
                head_front(4)
                head_front(5)
                norm_batch(1, 2)
                head_back(0)
                head_back(1)
                head_back(2)
                head_back(3)
                head_front(6)
                head_back(4)
                head_front(7)
                head_back(5)
                head_back(6)
                head_back(7)
                dh2_pools.append(d_half)


            with tc.tile_pool(name="po2", bufs=4, space="PSUM") as po2, \
                 tc.tile_pool(name="ob2", bufs=2) as ob2:
                dh2_pools[0]((0, 1, 2, 3), out_d, range(NJ), po2, ob2)
            if taps:
                for i in range(4):
                    nc.sync.dma_start(tap_d["d_qbf"][:, S * i:S * (i + 1)],
                                      qbf[i][:])
                    nc.sync.dma_start(tap_d["d_at"][:, S * i:S * (i + 1)],
                                      at_sb[i][:])
                nc.sync.dma_start(tap_d["d_kb"][:], kb_sb[:])
                nc.sync.dma_start(tap_d["d_vE"][:], vE_sb[:])

    nc.compile()
    return nc


def _host_prep(x, wq_w, wq_b, wk_w, wk_b, wv_w, wv_b, wo_w, wo_b, sinks):
    """Build per-core input maps (host-side sharding + layout prep)."""
    import ml_dtypes
    f = np.float32
    bf = ml_dtypes.bfloat16
    xm = x.reshape(S, DIM).astype(f)

    xP = np.zeros((128, DT * 1024), bf)
    for t in range(DT):
        dp = min(128, DIM - 128 * t)
        blk = xm[:, 128 * t:128 * t + dp].T.astype(bf)     # [dp, S]
        xP[:dp, 1024 * t:1024 * t + 512] = blk[:, 0:512]
        xP[:dp, 1024 * t + 512:1024 * (t + 1)] = blk[:, 512:1024]

    half = HD // 2
    inv_freq = 1.0 / (THETA ** (np.arange(half, dtype=np.float64) * 2.0 / HD))
    ang = np.arange(S, dtype=np.float64)[:, None] * inv_freq
    cos_t = np.cos(ang).T
    sin_t = np.sin(ang).T
    cos64 = np.concatenate([cos_t, cos_t], 0)              # [64, S]
    sin64 = np.concatenate([-sin_t, sin_t], 0)
    scale = HD ** -0.5
    cosq = (np.concatenate([cos64, cos64], 0) * scale).astype(bf)
    sinq = (np.concatenate([sin64, sin64], 0) * scale).astype(bf)
    cosk = cos64.astype(bf)
    sinkt = sin64.astype(bf)

    jj = np.arange(128)[:, None]
    ii = np.arange(256)[None, :]
    allow_l = (jj <= ii) & (ii < 128)
    allow_r = (ii >= 128) & (jj > ii - 128)
    mask256 = (allow_l | allow_r).astype(bf)
    maskB = np.concatenate([mask256, mask256], 1)          # [128, 512]

    id64 = np.tile(np.eye(64), (2, 1)).astype(bf)

    def tile_T(w, E):  # [E, DIM] -> tiled transposed [128, DT*E] bf16
        out = np.zeros((128, DT * E), bf)
        for t in range(DT):
            dp = min(128, DIM - 128 * t)
            out[:dp, E * t:E * (t + 1)] = \
                w[:, 128 * t:128 * t + dp].T.astype(bf)
        return out

    in_maps = []
    for c in range(NC):
        wq_c = wq_w[EL * c:EL * (c + 1)]                  # [512, 2880]
        wkv_c = np.concatenate([wk_w[HD * c:HD * (c + 1)],
                                wv_w[HD * c:HD * (c + 1)]], 0)  # [128, 2880]
        wo_c = np.ascontiguousarray(wo_w[:, EL * c:EL * (c + 1)].T)  # [512,2880]
        woP = np.zeros((128, 4 * DIM), bf)
        for et in range(4):
            woP[:, DIM * et:DIM * (et + 1)] = \
                wo_c[128 * et:128 * (et + 1)].astype(bf)
        es = np.exp(sinks[HL * c:HL * (c + 1)].astype(np.float64)).astype(f)
        esb4 = np.zeros((128, 4), f)
        for h in range(6):
            esb4[32 * (h % 4), h // 4] = es[h]
        esb4[64, 2] = es[6]
        esb4[64, 3] = es[7]
        in_maps.append({
            "xP": xP,
            "wqP": tile_T(wq_c, EL),
            "wkvP": tile_T(wkv_c, 128),
            "woP": woP,
            "qb": np.ascontiguousarray(
                wq_b[EL * c:EL * (c + 1)].reshape(4, 128).T).astype(f),
            "kvb": np.ascontiguousarray(np.concatenate(
                [wk_b[HD * c:HD * (c + 1)],
                 wv_b[HD * c:HD * (c + 1)]]).reshape(1, 128).T).astype(f),
            "cosq": cosq, "sinq": sinq, "cosk": cosk, "sinkt": sinkt,
            "maskB": maskB, "esb4": esb4, "id64": id64,
        })
    return in_maps


def run_on_hw(inputs, trace=False, taps=False, **kw):
    from concourse import bass_utils
    if "nc" not in _cache:
        _cache["nc"] = _build_module(taps=taps)
    in_maps = _host_prep(**inputs)
    res = bass_utils.run_bass_kernel_spmd(
        _cache["nc"], in_maps, core_ids=list(range(NC)), trace=trace, **kw)
    out = np.zeros((S, DIM), np.float64)
    for c in range(NC):
        out += np.asarray(res.results[c]["out"], dtype=np.float64)
    out = (out + inputs["wo_b"].astype(np.float64)).astype(np.float32)
    return out.reshape(B, S, DIM), res


def kernel(**inputs) -> np.ndarray:
    out, _ = run_on_hw(inputs, trace=False)
    return out
